# revision 1
# baseline (speedup 1.0000x reference)
"""C51 categorical-DQN histogram projection on Trainium2, 8-core data-parallel.

Exact reformulation of the reference scatter-add:
  m[b,j] = sum_a p[b,a] * hat(pos_ba - j),  hat(x) = relu(1 - |x|),
  pos_ba = clip(alpha_b + 0.99*a, 0, 50)  (alpha from reward/mask).
Per row, a 54-wide window of the in-row prefix-sum table P is fetched at a
data-dependent offset (indirect DMA, one offset per partition); window
diffs give atom masses; m = 3 shifted tent multiplies + clip corrections
on bins 0/50. mask=0 rows use a step table (all mass at a virtual atom 25).
"""
import sys
sys.path.insert(0, "/opt/trn_rl_repo")
import numpy as np
from concourse import bass, bacc, mybir, tile
from concourse.bass_utils import run_bass_kernel_spmd

F32 = mybir.dt.float32
I32 = mybir.dt.int32
OP = mybir.AluOpType
AF = mybir.ActivationFunctionType

P = 128
A = 51
B_TOTAL = 1048576
N_CORES = 8
BC = B_TOTAL // N_CORES
GAMMA = 0.99
ASTAR = 25
L, RP = 20, 20
SP = L + 52 + RP
SMIN, SMAX = -(RP - 1), L - 1
G = 32


def _host_consts():
    p = np.arange(P)[:, None]
    g = np.arange(G)[None, :]
    rowbase0 = ((g * P + p) * SP + (L - 1)).astype(np.int32)
    j001n = (-0.01 * np.arange(54, dtype=np.float32))[None, :].repeat(P, 0)
    return rowbase0, j001n


def _build_nc(Bc):
    TILE = P * G
    T = Bc // TILE
    FA = G * A

    nc = bacc.Bacc("TRN2", target_bir_lowering=False, debug=False)
    pr = nc.dram_tensor("pdist", [Bc, A], F32, kind="ExternalInput")
    rr = nc.dram_tensor("reward", [Bc], F32, kind="ExternalInput")
    mm = nc.dram_tensor("mask", [Bc], I32, kind="ExternalInput")
    rowbase_c = nc.dram_tensor("rowbase0", [P, G], I32, kind="ExternalInput")
    j001n_c = nc.dram_tensor("j001n", [P, 54], F32, kind="ExternalInput")
    mo = nc.dram_tensor("mout", [Bc, A], F32, kind="ExternalOutput")
    ptab = nc.dram_tensor("ptab", [Bc * SP, 1], F32, kind="Internal")

    prf = pr[:, :].rearrange("b a -> (b a)")
    mof = mo[:, :].rearrange("b a -> (b a)")
    ptf = ptab[:, :].rearrange("n o -> (n o)")

    def seg(flat, offset, *dims):
        return bass.AP(flat.tensor, offset, list(dims))

    with tile.TileContext(nc) as tc:
        with tc.tile_pool(name="const", bufs=1) as cpool:
            rowbase = cpool.tile([P, G], I32)
            nc.sync.dma_start(out=rowbase[:], in_=rowbase_c[:, :])
            j001n = cpool.tile([P, 54], F32)
            nc.sync.dma_start(out=j001n[:], in_=j001n_c[:, :])
            biases = []
            for k in range(3):
                bk = cpool.tile([P, 1], F32, tag=f"bias{k}")
                nc.vector.memset(bk[:], float(k))
                biases.append(bk)
            bone = cpool.tile([P, 1], F32, tag="bone")
            nc.vector.memset(bone[:], 1.0)
            zz = cpool.tile([P, FA], F32, tag="zz")
            nc.gpsimd.memset(zz[:], 0.0)

            with tc.tile_pool(name="sb", bufs=2) as pool:
                for t in range(T):
                    tbase = t * TILE
                    pt = pool.tile([P, FA], F32, tag="pt")
                    nc.sync.dma_start(
                        out=pt[:], in_=bass.AP(prf.tensor, tbase * A,
                                               [[A, P], [P * A, G], [1, A]]))
                    rt = pool.tile([P, G], F32, tag="rt")
                    nc.sync.dma_start(
                        out=rt[:], in_=bass.AP(rr[:].tensor, tbase, [[1, P], [P, G]]))
                    mkt = pool.tile([P, G], I32, tag="mkt")
                    nc.sync.dma_start(
                        out=mkt[:], in_=bass.AP(mm[:].tensor, tbase, [[1, P], [P, G]]))

                    # phase A: scan + mask=0 step + padded P-table write
                    st = pool.tile([P, 1 + FA], F32, tag="st")
                    nc.vector.memset(st[:, 0:1], 0.0)
                    nc.vector.tensor_tensor_scan(
                        out=st[:, 1:], data0=pt[:], data1=zz[:], initial=0.0,
                        op0=OP.add, op1=OP.add)
                    sth = st[:]

                    def stv(off, *dims):
                        return bass.AP(sth.tensor, sth.offset + off,
                                       [sth.ap[0]] + list(dims))
                    rowstart = stv(0, [A, G])
                    rowend = stv(A, [A, G])

                    notmk = pool.tile([P, G], I32, tag="notmk")
                    nc.vector.tensor_scalar(
                        out=notmk[:], in0=mkt[:], scalar1=1, scalar2=None,
                        op0=OP.bitwise_xor)
                    nc.vector.copy_predicated(
                        out=stv(1, [A, G], [1, ASTAR]),
                        mask=bass.AP(notmk[:].tensor, notmk[:].offset,
                                     [notmk[:].ap[0], [1, G], [0, ASTAR]]),
                        data=stv(0, [A, G], [0, ASTAR]))
                    nc.vector.copy_predicated(
                        out=stv(1 + ASTAR, [A, G], [1, 52 - 1 - ASTAR]),
                        mask=bass.AP(notmk[:].tensor, notmk[:].offset,
                                     [notmk[:].ap[0], [1, G], [0, 52 - 1 - ASTAR]]),
                        data=stv(A, [A, G], [0, 52 - 1 - ASTAR]))

                    nc.sync.dma_start(
                        out=seg(ptf, tbase * SP + L, [SP, P], [P * SP, G], [1, 52]),
                        in_=stv(0, [A, G], [1, 52]))
                    padLt = pool.tile([P, G * L], F32, tag="padLt")
                    nc.vector.tensor_copy(
                        out=bass.AP(padLt[:].tensor, padLt[:].offset,
                                    [padLt[:].ap[0], [L, G], [1, L]]),
                        in_=stv(0, [A, G], [0, L]))
                    nc.sync.dma_start(
                        out=seg(ptf, tbase * SP, [SP, P], [P * SP, G], [1, L]),
                        in_=padLt[:])
                    padRt = pool.tile([P, G * RP], F32, tag="padRt")
                    nc.vector.tensor_copy(
                        out=bass.AP(padRt[:].tensor, padRt[:].offset,
                                    [padRt[:].ap[0], [RP, G], [1, RP]]),
                        in_=stv(A, [A, G], [0, RP]))
                    nc.sync.dma_start(
                        out=seg(ptf, tbase * SP + L + 52, [SP, P], [P * SP, G], [1, RP]),
                        in_=padRt[:])

                    # phase B scalars
                    mf = pool.tile([P, G], F32, tag="mf")
                    nc.vector.tensor_copy(out=mf[:], in_=mkt[:])
                    a1 = pool.tile([P, G], F32, tag="a1")
                    nc.vector.tensor_scalar(out=a1[:], in0=rt[:], scalar1=2.5,
                                            scalar2=0.25, op0=OP.mult, op1=OP.add)
                    qt = pool.tile([P, G], F32, tag="qt")
                    nc.vector.tensor_scalar(out=qt[:], in0=rt[:], scalar1=2.5,
                                            scalar2=25.0, op0=OP.mult, op1=OP.add)
                    nc.vector.tensor_scalar(out=qt[:], in0=qt[:], scalar1=0.0,
                                            scalar2=50.0, op0=OP.max, op1=OP.min)
                    nc.vector.tensor_scalar(out=qt[:], in0=qt[:],
                                            scalar1=GAMMA * ASTAR, scalar2=None,
                                            op0=OP.subtract)
                    al = pool.tile([P, G], F32, tag="al")
                    nc.vector.tensor_tensor(out=al[:], in0=a1[:], in1=qt[:], op=OP.subtract)
                    nc.vector.tensor_tensor(out=al[:], in0=al[:], in1=mf[:], op=OP.mult)
                    nc.vector.tensor_tensor(out=al[:], in0=al[:], in1=qt[:], op=OP.add)
                    sf = pool.tile([P, G], F32, tag="sf")
                    nc.vector.tensor_scalar(out=sf[:], in0=al[:], scalar1=-0.5,
                                            scalar2=None, op0=OP.add)
                    si = pool.tile([P, G], I32, tag="si")
                    nc.vector.tensor_copy(out=si[:], in_=sf[:])
                    nc.vector.tensor_scalar(out=si[:], in0=si[:], scalar1=SMIN,
                                            scalar2=SMAX, op0=OP.max, op1=OP.min)
                    nc.vector.tensor_copy(out=sf[:], in_=si[:])
                    rho = pool.tile([P, G], F32, tag="rho")
                    nc.vector.tensor_scalar(out=rho[:], in0=sf[:], scalar1=-GAMMA,
                                            scalar2=-GAMMA, op0=OP.mult, op1=OP.add)
                    nc.vector.tensor_tensor(out=rho[:], in0=rho[:], in1=al[:], op=OP.add)
                    g0 = pool.tile([P, G], I32, tag="g0")
                    nc.vector.tensor_scalar(out=g0[:], in0=rowbase[:],
                                            scalar1=t * TILE * SP, scalar2=None,
                                            op0=OP.add)
                    nc.vector.tensor_tensor(out=g0[:], in0=g0[:], in1=si[:], op=OP.subtract)

                    W = pool.tile([P, G * 54], F32, tag="W")
                    for g in range(G):
                        nc.gpsimd.indirect_dma_start(
                            out=W[:, g * 54:(g + 1) * 54], out_offset=None,
                            in_=ptab[:, :],
                            in_offset=bass.IndirectOffsetOnAxis(
                                ap=g0[:, g:g + 1], axis=0))
                    Wh = W[:]

                    def wv(off, *dims):
                        return bass.AP(Wh.tensor, Wh.offset + off,
                                       [Wh.ap[0]] + list(dims))
                    wd = pool.tile([P, G * 53], F32, tag="wd")
                    nc.vector.tensor_tensor(
                        out=wd[:], in0=wv(1, [54, G], [1, 53]),
                        in1=wv(0, [54, G], [1, 53]), op=OP.subtract)
                    wdh = wd[:]

                    def wdv(off, *dims):
                        return bass.AP(wdh.tensor, wdh.offset + off,
                                       [wdh.ap[0]] + list(dims))
                    Y = pool.tile([P, G * 54], F32, tag="Y")
                    nc.vector.tensor_tensor(
                        out=Y[:],
                        in0=bass.AP(rho[:].tensor, rho[:].offset,
                                    [rho[:].ap[0], [1, G], [0, 54]]),
                        in1=bass.AP(j001n[:].tensor, j001n[:].offset,
                                    [j001n[:].ap[0], [0, G], [1, 54]]),
                        op=OP.add)
                    Yh = Y[:]

                    def yv(off, *dims):
                        return bass.AP(Yh.tensor, Yh.offset + off,
                                       [Yh.ap[0]] + list(dims))

                    mt_ = pool.tile([P, FA], F32, tag="mt_")
                    au = pool.tile([P, FA], F32, tag="au")
                    tmp = pool.tile([P, FA], F32, tag="tmp")
                    for k in range(3):
                        nc.scalar.activation(
                            out=au[:], in_=yv(k, [54, G], [1, A]),
                            func=AF.Abs, bias=biases[k][:], scale=1.0)
                        nc.scalar.activation(
                            out=au[:], in_=au[:], func=AF.Relu, bias=bone[:], scale=-1.0)
                        if k == 0:
                            nc.vector.tensor_tensor(
                                out=mt_[:], in0=au[:], in1=wdv(0, [53, G], [1, A]),
                                op=OP.mult)
                        else:
                            nc.vector.tensor_tensor(
                                out=tmp[:], in0=au[:], in1=wdv(k, [53, G], [1, A]),
                                op=OP.mult)
                            nc.vector.tensor_tensor(
                                out=mt_[:], in0=mt_[:], in1=tmp[:], op=OP.add)

                    d0 = pool.tile([P, G], F32, tag="d0")
                    nc.vector.tensor_tensor(out=d0[:], in0=wv(0, [54, G]),
                                            in1=rowstart, op=OP.subtract)
                    cx = pool.tile([P, G], F32, tag="cx")
                    t2 = pool.tile([P, G], F32, tag="t2")
                    for i in (0, 1):
                        nc.vector.tensor_scalar(out=cx[:], in0=rho[:], scalar1=-1.0,
                                                scalar2=-GAMMA * i, op0=OP.mult,
                                                op1=OP.add)
                        nc.vector.tensor_scalar(out=cx[:], in0=cx[:], scalar1=0.0,
                                                scalar2=1.0, op0=OP.max, op1=OP.min)
                        nc.vector.tensor_tensor(out=t2[:], in0=cx[:],
                                                in1=wdv(i, [53, G]), op=OP.mult)
                        nc.vector.tensor_tensor(out=d0[:], in0=d0[:], in1=t2[:],
                                                op=OP.add)
                    d5 = pool.tile([P, G], F32, tag="d5")
                    nc.vector.tensor_tensor(out=d5[:], in0=rowend,
                                            in1=wv(53, [54, G]), op=OP.subtract)
                    for i in (50, 51, 52):
                        nc.vector.tensor_scalar(out=cx[:], in0=rho[:],
                                                scalar1=GAMMA * i - 50.0, scalar2=None,
                                                op0=OP.add)
                        nc.vector.tensor_scalar(out=cx[:], in0=cx[:], scalar1=0.0,
                                                scalar2=1.0, op0=OP.max, op1=OP.min)
                        nc.vector.tensor_tensor(out=t2[:], in0=cx[:],
                                                in1=wdv(i, [53, G]), op=OP.mult)
                        nc.vector.tensor_tensor(out=d5[:], in0=d5[:], in1=t2[:],
                                                op=OP.add)
                    mh = mt_[:]
                    nc.vector.tensor_tensor(
                        out=bass.AP(mh.tensor, mh.offset, [mh.ap[0], [A, G]]),
                        in0=bass.AP(mh.tensor, mh.offset, [mh.ap[0], [A, G]]),
                        in1=d0[:], op=OP.add)
                    nc.vector.tensor_tensor(
                        out=bass.AP(mh.tensor, mh.offset + 50, [mh.ap[0], [A, G]]),
                        in0=bass.AP(mh.tensor, mh.offset + 50, [mh.ap[0], [A, G]]),
                        in1=d5[:], op=OP.add)

                    nc.sync.dma_start(
                        out=bass.AP(mof.tensor, tbase * A, [[A, P], [P * A, G], [1, A]]),
                        in_=mt_[:])
    nc.compile()
    return nc


_NC_CACHE = {}


def kernel(batch_reward, max_next_dist, supports, non_final_mask):
    assert max_next_dist.shape == (B_TOTAL, A)
    if "nc" not in _NC_CACHE:
        _NC_CACHE["nc"] = _build_nc(BC)
    nc = _NC_CACHE["nc"]
    rowbase0, j001n = _host_consts()
    in_maps = []
    for c in range(N_CORES):
        s = slice(c * BC, (c + 1) * BC)
        in_maps.append({
            "pdist": np.ascontiguousarray(max_next_dist[s]).astype(np.float32),
            "reward": np.ascontiguousarray(batch_reward[s]).astype(np.float32),
            "mask": np.ascontiguousarray(non_final_mask[s]).astype(np.int32),
            "rowbase0": rowbase0,
            "j001n": j001n,
        })
    res = run_bass_kernel_spmd(nc, in_maps, core_ids=list(range(N_CORES)))
    return np.concatenate([res.results[c]["mout"] for c in range(N_CORES)], axis=0)



# revision 13
# speedup vs baseline: 38.4811x; 38.4811x over previous
"""C51 categorical-DQN histogram projection on Trainium2, 8-core data-parallel.

Direct-shift reformulation (no DRAM prefix table, no indirect DMA):
  m[b,j] = sum_k au_k[j] * pS[j+k], k in {-1,0,1}
  pS[n] = p_eff[n - s] (zero outside), s = floor((alpha+0.48)/gamma),
  au_k[j] = relu(1 - |rho + gamma*k - 0.01 j|), rho = alpha - gamma*s in
  [-0.48, 0.51].  The per-row integer shift s is applied in-SBUF by a
  two-level (base-5) cascade of copy_predicated selects over a zero-padded
  per-row copy of p (bf16).  The three tap weights are computed in ONE
  activation pair over a 3-section table (Y3 = rho - 0.01 j + gamma k).
  Edge bins are overwritten exactly (both windows in one op via two-window
  access patterns):
    m[0]  = sum_{a<16}  p[a] * clip(1 - alpha - g a, 0, 1) * mask
    m[50] = sum_{a>=35} p[a] * clip(alpha + g a - 49, 0, 1) * mask
  mask=0 rows use p_eff = onehot(25), alpha = q - 24.75 (q = clip(2.5r+25)).
Row mapping: row = p*1024 + t*G + g (contiguous per-partition scalars).
Note: f32->i32 tensor_copy ROUNDS on hardware (trunc in CoreSim); the
shift digits are derived with pure integer ops so both agree, and the
floor() offsets tolerate +-1 (the rho window [-0.48, 0.98] absorbs it).
"""
import sys
sys.path.insert(0, "/opt/trn_rl_repo")
import numpy as np
from concourse import bass, bacc, mybir, tile
from concourse.bass_utils import run_bass_kernel_spmd

F32 = mybir.dt.float32
BF16 = mybir.dt.bfloat16
I32 = mybir.dt.int32
I16 = mybir.dt.int16
OP = mybir.AluOpType
AF = mybir.ActivationFunctionType

P = 128
A = 51
B_TOTAL = 1048576
N_CORES = 8
BC = B_TOTAL // N_CORES
G = 64
TILE = P * G
T = BC // TILE          # 16 tiles/core
TG = BC // P            # 1024 scalars per partition
GAMMA = 0.99
PADL = 17
WPB = 85                # padded p width per group: zeros outside [17,68)
WT1 = 58                # coarse-select output width
WT1S = 60               # T1 storage stride
WPS = 53                # fine-select output width (pS[n], n in [-1,51])
WPSS = 54               # pS storage stride
EW = 16                 # edge-window atoms per side
W3 = 3 * A              # 3-section tap table width


def _host_consts():
    j = np.arange(A, dtype=np.float32)
    j3 = np.concatenate([-0.01 * j + GAMMA * k for k in (-1, 0, 1)])
    j3 = j3[None, :].repeat(P, 0).astype(np.float32)
    tabL = -GAMMA * np.arange(EW, dtype=np.float32)
    tabR = GAMMA * (35.0 + np.arange(EW, dtype=np.float32)) - 48.0
    tabLR = np.concatenate([tabL, tabR])[None, :].repeat(P, 0).astype(
        np.float32)
    return j3, tabLR


def _build_nc(Bc, repeat=1):
    from contextlib import nullcontext
    FA = G * A
    T_ = Bc // TILE
    TG_ = Bc // P
    nc = bacc.Bacc("TRN2", target_bir_lowering=False, debug=False)
    pr = nc.dram_tensor("pdist", [Bc, A], F32, kind="ExternalInput")
    rr = nc.dram_tensor("reward", [Bc], F32, kind="ExternalInput")
    mm = nc.dram_tensor("mask", [Bc], I32, kind="ExternalInput")
    j3_c = nc.dram_tensor("j3", [P, W3], F32, kind="ExternalInput")
    tab_c = nc.dram_tensor("tabLR", [P, 2 * EW], F32, kind="ExternalInput")
    mo = nc.dram_tensor("mout", [Bc, A], F32, kind="ExternalOutput")

    prf = pr[:, :].rearrange("b a -> (b a)")
    mof = mo[:, :].rearrange("b a -> (b a)")

    with tile.TileContext(nc) as tc:
      with (tc.For_i(0, repeat) if repeat > 1 else nullcontext()):
        with tc.tile_pool(name="const", bufs=1) as cpool:
            j3 = cpool.tile([P, W3], F32)
            nc.sync.dma_start(out=j3[:], in_=j3_c[:, :])
            tabLR = cpool.tile([P, 2 * EW], F32)
            nc.sync.dma_start(out=tabLR[:], in_=tab_c[:, :])
            bone = cpool.tile([P, 1], F32, tag="bone")
            nc.vector.memset(bone[:], 1.0)

            # ---- prepass: all per-row scalars for the whole core ----
            mfv = cpool.tile([P, TG_], F32, tag="mfv")
            rho = cpool.tile([P, TG_], F32, tag="rho")
            pnq = cpool.tile([P, TG_ * 2], F32, tag="pnq")
            nm16 = cpool.tile([P, TG_], BF16, tag="nm16")
            mcs = {c: cpool.tile([P, TG_], I16, tag=f"mc{c}", name=f"mc{c}")
                   for c in range(1, 6)}
            mfs = {f: cpool.tile([P, TG_], I16, tag=f"mf{f}", name=f"mf{f}")
                   for f in range(1, 5)}
            w0 = cpool.tile([P, TG_], F32, tag="w0")
            w1 = cpool.tile([P, TG_], F32, tag="w1")
            w2 = cpool.tile([P, TG_], F32, tag="w2")
            si = cpool.tile([P, TG_], I32, tag="si")
            ci = cpool.tile([P, TG_], I32, tag="ci")
            fi = cpool.tile([P, TG_], I32, tag="fi")

            nc.sync.dma_start(out=w0[:], in_=bass.AP(rr[:].tensor, 0,
                                                     [[TG_, P], [1, TG_]]))
            nc.sync.dma_start(out=si[:], in_=bass.AP(mm[:].tensor, 0,
                                                     [[TG_, P], [1, TG_]]))
            nc.gpsimd.tensor_copy(out=mfv[:], in_=si[:])
            nc.gpsimd.tensor_scalar(out=w2[:], in0=mfv[:], scalar1=-1.0,
                                    scalar2=1.0, op0=OP.mult, op1=OP.add)
            nc.gpsimd.tensor_copy(out=nm16[:], in_=w2[:])
            # w2 = aq = clip(2.5r+25, 0, 50) - 24.75
            nc.gpsimd.tensor_scalar(out=w1[:], in0=w0[:], scalar1=2.5,
                                    scalar2=25.0, op0=OP.mult, op1=OP.add)
            nc.gpsimd.tensor_scalar(out=w1[:], in0=w1[:], scalar1=0.0,
                                    scalar2=50.0, op0=OP.max, op1=OP.min)
            nc.gpsimd.tensor_scalar(out=w2[:], in0=w1[:], scalar1=-24.75,
                                    scalar2=None, op0=OP.add)
            # w1 = al = mf*(a1 - aq) + aq
            nc.gpsimd.tensor_scalar(out=w1[:], in0=w0[:], scalar1=2.5,
                                    scalar2=0.25, op0=OP.mult, op1=OP.add)
            nc.gpsimd.tensor_tensor(out=w1[:], in0=w1[:], in1=w2[:],
                                    op=OP.subtract)
            nc.gpsimd.tensor_tensor(out=w1[:], in0=w1[:], in1=mfv[:],
                                    op=OP.mult)
            nc.gpsimd.tensor_tensor(out=w1[:], in0=w1[:], in1=w2[:],
                                    op=OP.add)
            # pnq interleaved (oma, -oma): oma = 1 - al
            nc.gpsimd.tensor_scalar(out=w0[:], in0=w1[:], scalar1=-1.0,
                                    scalar2=1.0, op0=OP.mult, op1=OP.add)
            pq = pnq[:]
            nc.gpsimd.tensor_copy(
                out=bass.AP(pq.tensor, pq.offset, [pq.ap[0], [2, TG_]]),
                in_=w0[:])
            nc.gpsimd.tensor_scalar(out=w0[:], in0=w0[:], scalar1=-1.0,
                                    scalar2=None, op0=OP.mult)
            nc.gpsimd.tensor_copy(
                out=bass.AP(pq.tensor, pq.offset + 1, [pq.ap[0], [2, TG_]]),
                in_=w0[:])
            # si = round((al+0.48)/g + 16 - 0.5); rho = al - 0.99*si + 15.84
            nc.gpsimd.tensor_scalar(out=w0[:], in0=w1[:],
                                    scalar1=1.0101010101010102,
                                    scalar2=15.984848484848484,
                                    op0=OP.mult, op1=OP.add)
            nc.gpsimd.tensor_copy(out=si[:], in_=w0[:])
            nc.gpsimd.tensor_copy(out=w2[:], in_=si[:])
            nc.gpsimd.tensor_scalar(out=w2[:], in0=w2[:], scalar1=-GAMMA,
                                    scalar2=15.84, op0=OP.mult, op1=OP.add)
            nc.gpsimd.tensor_tensor(out=rho[:], in0=w2[:], in1=w1[:],
                                    op=OP.add)
            # si := delta = 32 - si in [2,29]; coarse by is_ge cascade,
            # c = sum(masks), f = delta - 5c in [0,4] (pure integer).
            nc.gpsimd.tensor_scalar(out=si[:], in0=si[:], scalar1=-1,
                                    scalar2=32, op0=OP.mult, op1=OP.add)
            for c in range(1, 6):
                nc.gpsimd.tensor_scalar(out=fi[:], in0=si[:],
                                        scalar1=5 * c, scalar2=None,
                                        op0=OP.is_ge)
                nc.gpsimd.tensor_copy(out=mcs[c][:], in_=fi[:])
                if c == 1:
                    nc.gpsimd.tensor_copy(out=ci[:], in_=fi[:])
                else:
                    nc.gpsimd.tensor_tensor(out=ci[:], in0=ci[:],
                                            in1=fi[:], op=OP.add)
            nc.gpsimd.tensor_scalar(out=fi[:], in0=ci[:], scalar1=-5,
                                    scalar2=None, op0=OP.mult)
            nc.gpsimd.tensor_tensor(out=fi[:], in0=fi[:], in1=si[:],
                                    op=OP.add)
            for f in range(1, 5):
                nc.gpsimd.tensor_scalar(out=ci[:], in0=fi[:], scalar1=f,
                                        scalar2=None, op0=OP.is_equal)
                nc.gpsimd.tensor_copy(out=mfs[f][:], in_=ci[:])

            def gview(tl, t, w):
                h = tl[:]
                return bass.AP(h.tensor, h.offset + t * G,
                               [h.ap[0], [1, G], [0, w]])

            with tc.tile_pool(name="io", bufs=2) as iop, \
                 tc.tile_pool(name="wk2", bufs=2) as wk2, \
                 tc.tile_pool(name="wk1", bufs=1) as wk1:
                for t in range(T_):
                    pt = iop.tile([P, FA], F32, tag="pt")
                    nc.sync.dma_start(
                        out=pt[:],
                        in_=bass.AP(prf.tensor, t * G * A,
                                    [[TG_ * A, P], [A, G], [1, A]]))

                    # padded p_eff (bf16), single two-window pad memset
                    PB = wk2.tile([P, G * WPB], BF16, tag="PB")

                    def pbv(off, w):
                        h = PB[:]
                        return bass.AP(h.tensor, h.offset + off,
                                       [h.ap[0], [WPB, G], [1, w]])

                    pbh = PB[:]
                    nc.gpsimd.memset(
                        bass.AP(pbh.tensor, pbh.offset,
                                [pbh.ap[0], [WPB, G], [PADL + A, 2],
                                 [1, PADL]]), 0.0)
                    nc.gpsimd.tensor_tensor(
                        out=pbv(PADL, A), in0=pt[:], in1=gview(mfv, t, A),
                        op=OP.mult)
                    pbcol = bass.AP(pbh.tensor, pbh.offset + PADL + 25,
                                    [pbh.ap[0], [WPB, G]])
                    nc.gpsimd.tensor_tensor(
                        out=pbcol, in0=pbcol,
                        in1=bass.AP(nm16[:].tensor, nm16[:].offset + t * G,
                                    [nm16[:].ap[0], [1, G]]), op=OP.add)

                    # coarse select: T1[x] = PB[x + 5c]
                    T1 = wk1.tile([P, G * WT1S], BF16, tag="T1")

                    def t1v(off, w):
                        h = T1[:]
                        return bass.AP(h.tensor, h.offset + off,
                                       [h.ap[0], [WT1S, G], [1, w]])

                    nc.vector.tensor_copy(out=t1v(0, WT1), in_=pbv(0, WT1))
                    for c in range(1, 6):
                        nc.vector.copy_predicated(
                            out=t1v(0, WT1), mask=gview(mcs[c], t, WT1),
                            data=pbv(5 * c, WT1))
                    # fine select: pS[x] = T1[x + f]
                    pS = wk1.tile([P, G * WPSS], BF16, tag="pS")

                    def psv(off, w):
                        h = pS[:]
                        return bass.AP(h.tensor, h.offset + off,
                                       [h.ap[0], [WPSS, G], [1, w]])

                    nc.vector.tensor_copy(out=psv(0, WPS), in_=t1v(0, WPS))
                    for f in range(1, 5):
                        nc.vector.copy_predicated(
                            out=psv(0, WPS), mask=gview(mfs[f], t, WPS),
                            data=t1v(f, WPS))

                    # tap weights: au3 = relu(1 - |rho - 0.01 j + g k|),
                    # all 3 sections in one op pair
                    au3 = wk1.tile([P, G * W3], BF16, tag="au3")
                    nc.vector.tensor_tensor(
                        out=au3[:], in0=gview(rho, t, W3),
                        in1=bass.AP(j3[:].tensor, j3[:].offset,
                                    [j3[:].ap[0], [0, G], [1, W3]]),
                        op=OP.add)
                    nc.scalar.activation(out=au3[:], in_=au3[:], func=AF.Abs,
                                         bias=0.0, scale=1.0)
                    nc.scalar.activation(out=au3[:], in_=au3[:], func=AF.Relu,
                                         bias=bone[:], scale=-1.0)

                    def auv(k):
                        h = au3[:]
                        return bass.AP(h.tensor, h.offset + (k + 1) * A,
                                       [h.ap[0], [W3, G], [1, A]])

                    # taps (bf16 accumulate, final add widens to f32)
                    mt = iop.tile([P, FA], F32, tag="mt")
                    tm1 = wk1.tile([P, FA], BF16, tag="tm1")
                    tm2 = wk1.tile([P, FA], BF16, tag="tm2")
                    nc.vector.tensor_tensor(out=tm1[:], in0=auv(-1),
                                            in1=psv(0, A), op=OP.mult)
                    nc.vector.tensor_tensor(out=tm2[:], in0=auv(0),
                                            in1=psv(1, A), op=OP.mult)
                    nc.vector.tensor_tensor(out=tm1[:], in0=tm1[:],
                                            in1=tm2[:], op=OP.add)
                    nc.vector.tensor_tensor(out=tm2[:], in0=auv(1),
                                            in1=psv(2, A), op=OP.mult)
                    nc.vector.tensor_tensor(out=mt[:], in0=tm1[:],
                                            in1=tm2[:], op=OP.add)

                    # edges: both windows in one op chain
                    wc = wk1.tile([P, G * 2 * EW], F32, tag="wc")

                    def wcv(dims):
                        h = wc[:]
                        return bass.AP(h.tensor, h.offset, [h.ap[0]] + dims)

                    nc.gpsimd.tensor_tensor(
                        out=wcv([[2 * EW, G], [1, 2 * EW]]),
                        in0=bass.AP(tabLR[:].tensor, tabLR[:].offset,
                                    [tabLR[:].ap[0], [0, G], [1, 2 * EW]]),
                        in1=bass.AP(pnq[:].tensor, pnq[:].offset + t * 2 * G,
                                    [pnq[:].ap[0], [2, G], [1, 2], [0, EW]]),
                        op=OP.add)
                    nc.gpsimd.tensor_scalar(out=wc[:], in0=wc[:], scalar1=0.0,
                                            scalar2=1.0, op0=OP.max,
                                            op1=OP.min)
                    nc.gpsimd.tensor_tensor(
                        out=wcv([[2 * EW, G], [1, 2 * EW]]),
                        in0=wcv([[2 * EW, G], [1, 2 * EW]]),
                        in1=gview(mfv, t, 2 * EW), op=OP.mult)
                    pth = pt[:]
                    nc.gpsimd.tensor_tensor(
                        out=wcv([[2 * EW, G], [1, 2 * EW]]),
                        in0=wcv([[2 * EW, G], [1, 2 * EW]]),
                        in1=bass.AP(pth.tensor, pth.offset,
                                    [pth.ap[0], [A, G], [35, 2], [1, EW]]),
                        op=OP.mult)
                    m05 = wk1.tile([P, G * 2], F32, tag="m05")
                    nc.vector.tensor_reduce(
                        out=bass.AP(m05[:].tensor, m05[:].offset,
                                    [m05[:].ap[0], [2, G], [1, 2]]),
                        in_=wcv([[2 * EW, G], [EW, 2], [1, EW]]),
                        axis=mybir.AxisListType.X, op=OP.add)
                    mth = mt[:]
                    nc.gpsimd.tensor_copy(
                        out=bass.AP(mth.tensor, mth.offset,
                                    [mth.ap[0], [A, G], [50, 2]]),
                        in_=bass.AP(m05[:].tensor, m05[:].offset,
                                    [m05[:].ap[0], [2, G], [1, 2]]))

                    nc.sync.dma_start(
                        out=bass.AP(mof.tensor, t * G * A,
                                    [[TG_ * A, P], [A, G], [1, A]]),
                        in_=mt[:])
    nc.compile()
    return nc


_NC_CACHE = {}


def kernel(batch_reward, max_next_dist, supports, non_final_mask):
    assert max_next_dist.shape == (B_TOTAL, A)
    if "nc" not in _NC_CACHE:
        _NC_CACHE["nc"] = _build_nc(BC)
    nc = _NC_CACHE["nc"]
    j3, tabLR = _host_consts()
    in_maps = []
    for c in range(N_CORES):
        s = slice(c * BC, (c + 1) * BC)
        in_maps.append({
            "pdist": np.ascontiguousarray(max_next_dist[s]).astype(np.float32),
            "reward": np.ascontiguousarray(batch_reward[s]).astype(np.float32),
            "mask": np.ascontiguousarray(non_final_mask[s]).astype(np.int32),
            "j3": j3, "tabLR": tabLR,
        })
    res = run_bass_kernel_spmd(nc, in_maps, core_ids=list(range(N_CORES)))
    return np.concatenate([res.results[c]["mout"] for c in range(N_CORES)],
                          axis=0)


# revision 14
# speedup vs baseline: 59.5221x; 1.5468x over previous
"""C51 categorical-DQN histogram projection on Trainium2, 8-core data-parallel.

Direct-shift reformulation (no DRAM prefix table, no indirect DMA):
  m[b,j] = sum_k au_k[j] * pS[j+k], k in {-1,0,1}
  pS[n] = p_eff[n - s] (zero outside), s = floor((alpha+0.48)/gamma),
  au_k[j] = relu(1 - |rho + gamma*k - 0.01 j|), rho = alpha - gamma*s in
  [-0.48, 0.51].  The per-row integer shift s is applied in-SBUF by a
  two-level (base-5) cascade of copy_predicated selects over a zero-padded
  per-row copy of p (bf16).  The three tap weights are computed in ONE
  activation pair over a 3-section table (Y3 = rho - 0.01 j + gamma k).
  Edge bins are overwritten exactly (both windows in one op via two-window
  access patterns):
    m[0]  = sum_{a<16}  p[a] * clip(1 - alpha - g a, 0, 1) * mask
    m[50] = sum_{a>=35} p[a] * clip(alpha + g a - 49, 0, 1) * mask
  mask=0 rows use p_eff = onehot(25), alpha = q - 24.75 (q = clip(2.5r+25)).
Row mapping: row = p*1024 + t*G + g (contiguous per-partition scalars).
Note: f32->i32 tensor_copy ROUNDS on hardware (trunc in CoreSim); the
shift digits are derived with pure integer ops so both agree, and the
floor() offsets tolerate +-1 (the rho window [-0.48, 0.98] absorbs it).
"""
import sys
sys.path.insert(0, "/opt/trn_rl_repo")
import numpy as np
from concourse import bass, bacc, mybir, tile
from concourse.bass_utils import run_bass_kernel_spmd

F32 = mybir.dt.float32
BF16 = mybir.dt.bfloat16
I32 = mybir.dt.int32
I16 = mybir.dt.int16
OP = mybir.AluOpType
AF = mybir.ActivationFunctionType

P = 128
A = 51
B_TOTAL = 1048576
N_CORES = 8
BC = B_TOTAL // N_CORES
G = 64
TILE = P * G
T = BC // TILE          # 16 tiles/core
TG = BC // P            # 1024 scalars per partition
GAMMA = 0.99
PADL = 17
WPB = 85                # padded p width per group: zeros outside [17,68)
WT1 = 58                # coarse-select output width
WT1S = 60               # T1 storage stride
WPS = 53                # fine-select output width (pS[n], n in [-1,51])
WPSS = 54               # pS storage stride
EW = 16                 # edge-window atoms per side
W3 = 3 * A              # 3-section tap table width


def _host_consts():
    j = np.arange(A, dtype=np.float32)
    j3 = np.concatenate([-0.01 * j + GAMMA * k for k in (-1, 0, 1)])
    j3 = j3[None, :].repeat(P, 0).astype(np.float32)
    tabL = -GAMMA * np.arange(EW, dtype=np.float32)
    tabR = GAMMA * (35.0 + np.arange(EW, dtype=np.float32)) - 48.0
    import ml_dtypes
    tabLR = np.concatenate([tabL, tabR])[None, :].repeat(P, 0).astype(
        ml_dtypes.bfloat16)
    return j3, tabLR


def _build_nc(Bc, repeat=1):
    from contextlib import nullcontext
    FA = G * A
    T_ = Bc // TILE
    TG_ = Bc // P
    nc = bacc.Bacc("TRN2", target_bir_lowering=False, debug=False)
    pr = nc.dram_tensor("pdist", [Bc, A], F32, kind="ExternalInput")
    rr = nc.dram_tensor("reward", [Bc], F32, kind="ExternalInput")
    mm = nc.dram_tensor("mask", [Bc], I32, kind="ExternalInput")
    j3_c = nc.dram_tensor("j3", [P, W3], F32, kind="ExternalInput")
    tab_c = nc.dram_tensor("tabLR", [P, 2 * EW], BF16, kind="ExternalInput")
    mo = nc.dram_tensor("mout", [Bc, A], F32, kind="ExternalOutput")

    prf = pr[:, :].rearrange("b a -> (b a)")
    mof = mo[:, :].rearrange("b a -> (b a)")

    with tile.TileContext(nc) as tc:
      with (tc.For_i(0, repeat) if repeat > 1 else nullcontext()):
        with tc.tile_pool(name="const", bufs=1) as cpool:
            j3 = cpool.tile([P, W3], F32)
            nc.sync.dma_start(out=j3[:], in_=j3_c[:, :])
            tabLR = cpool.tile([P, 2 * EW], BF16)
            nc.sync.dma_start(out=tabLR[:], in_=tab_c[:, :])
            bone = cpool.tile([P, 1], F32, tag="bone")
            nc.vector.memset(bone[:], 1.0)

            # ---- prepass: all per-row scalars for the whole core ----
            mfv = cpool.tile([P, TG_], F32, tag="mfv")
            rho = cpool.tile([P, TG_], F32, tag="rho")
            pnq = cpool.tile([P, TG_ * 2], BF16, tag="pnq")
            nm16 = cpool.tile([P, TG_], BF16, tag="nm16")
            mcs = {c: cpool.tile([P, TG_], I16, tag=f"mc{c}", name=f"mc{c}")
                   for c in range(1, 6)}
            mfs = {f: cpool.tile([P, TG_], I16, tag=f"mf{f}", name=f"mf{f}")
                   for f in range(1, 5)}
            w0 = cpool.tile([P, TG_], F32, tag="w0")
            w1 = cpool.tile([P, TG_], F32, tag="w1")
            w2 = cpool.tile([P, TG_], F32, tag="w2")
            si = cpool.tile([P, TG_], I32, tag="si")
            ci = cpool.tile([P, TG_], I32, tag="ci")
            fi = cpool.tile([P, TG_], I32, tag="fi")

            nc.sync.dma_start(out=w0[:], in_=bass.AP(rr[:].tensor, 0,
                                                     [[TG_, P], [1, TG_]]))
            nc.sync.dma_start(out=si[:], in_=bass.AP(mm[:].tensor, 0,
                                                     [[TG_, P], [1, TG_]]))
            nc.gpsimd.tensor_copy(out=mfv[:], in_=si[:])
            nc.gpsimd.tensor_scalar(out=w2[:], in0=mfv[:], scalar1=-1.0,
                                    scalar2=1.0, op0=OP.mult, op1=OP.add)
            nc.gpsimd.tensor_copy(out=nm16[:], in_=w2[:])
            # w2 = aq = clip(2.5r+25, 0, 50) - 24.75
            nc.gpsimd.tensor_scalar(out=w1[:], in0=w0[:], scalar1=2.5,
                                    scalar2=25.0, op0=OP.mult, op1=OP.add)
            nc.gpsimd.tensor_scalar(out=w1[:], in0=w1[:], scalar1=0.0,
                                    scalar2=50.0, op0=OP.max, op1=OP.min)
            nc.gpsimd.tensor_scalar(out=w2[:], in0=w1[:], scalar1=-24.75,
                                    scalar2=None, op0=OP.add)
            # w1 = al = mf*(a1 - aq) + aq
            nc.gpsimd.tensor_scalar(out=w1[:], in0=w0[:], scalar1=2.5,
                                    scalar2=0.25, op0=OP.mult, op1=OP.add)
            nc.gpsimd.tensor_tensor(out=w1[:], in0=w1[:], in1=w2[:],
                                    op=OP.subtract)
            nc.gpsimd.tensor_tensor(out=w1[:], in0=w1[:], in1=mfv[:],
                                    op=OP.mult)
            nc.gpsimd.tensor_tensor(out=w1[:], in0=w1[:], in1=w2[:],
                                    op=OP.add)
            # pnq interleaved (oma, -oma): oma = 1 - al
            nc.gpsimd.tensor_scalar(out=w0[:], in0=w1[:], scalar1=-1.0,
                                    scalar2=1.0, op0=OP.mult, op1=OP.add)
            pq = pnq[:]
            nc.gpsimd.tensor_copy(
                out=bass.AP(pq.tensor, pq.offset, [pq.ap[0], [2, TG_]]),
                in_=w0[:])
            nc.gpsimd.tensor_scalar(out=w0[:], in0=w0[:], scalar1=-1.0,
                                    scalar2=None, op0=OP.mult)
            nc.gpsimd.tensor_copy(
                out=bass.AP(pq.tensor, pq.offset + 1, [pq.ap[0], [2, TG_]]),
                in_=w0[:])
            # si = round((al+0.48)/g + 16 - 0.5); rho = al - 0.99*si + 15.84
            nc.gpsimd.tensor_scalar(out=w0[:], in0=w1[:],
                                    scalar1=1.0101010101010102,
                                    scalar2=15.984848484848484,
                                    op0=OP.mult, op1=OP.add)
            nc.gpsimd.tensor_copy(out=si[:], in_=w0[:])
            nc.gpsimd.tensor_copy(out=w2[:], in_=si[:])
            nc.gpsimd.tensor_scalar(out=w2[:], in0=w2[:], scalar1=-GAMMA,
                                    scalar2=15.84, op0=OP.mult, op1=OP.add)
            nc.gpsimd.tensor_tensor(out=rho[:], in0=w2[:], in1=w1[:],
                                    op=OP.add)
            # si := delta = 32 - si in [2,29]; coarse by is_ge cascade,
            # c = sum(masks), f = delta - 5c in [0,4] (pure integer).
            nc.gpsimd.tensor_scalar(out=si[:], in0=si[:], scalar1=-1,
                                    scalar2=32, op0=OP.mult, op1=OP.add)
            for c in range(1, 6):
                nc.gpsimd.tensor_scalar(out=fi[:], in0=si[:],
                                        scalar1=5 * c, scalar2=None,
                                        op0=OP.is_ge)
                nc.gpsimd.tensor_copy(out=mcs[c][:], in_=fi[:])
                if c == 1:
                    nc.gpsimd.tensor_copy(out=ci[:], in_=fi[:])
                else:
                    nc.gpsimd.tensor_tensor(out=ci[:], in0=ci[:],
                                            in1=fi[:], op=OP.add)
            nc.gpsimd.tensor_scalar(out=fi[:], in0=ci[:], scalar1=-5,
                                    scalar2=None, op0=OP.mult)
            nc.gpsimd.tensor_tensor(out=fi[:], in0=fi[:], in1=si[:],
                                    op=OP.add)
            for f in range(1, 5):
                nc.gpsimd.tensor_scalar(out=ci[:], in0=fi[:], scalar1=f,
                                        scalar2=None, op0=OP.is_equal)
                nc.gpsimd.tensor_copy(out=mfs[f][:], in_=ci[:])

            def gview(tl, t, w):
                h = tl[:]
                return bass.AP(h.tensor, h.offset + t * G,
                               [h.ap[0], [1, G], [0, w]])

            with tc.tile_pool(name="io", bufs=2) as iop, \
                 tc.tile_pool(name="wk2", bufs=2) as wk2, \
                 tc.tile_pool(name="wk1", bufs=1) as wk1:
                for t in range(T_):
                    pt = iop.tile([P, FA], F32, tag="pt")
                    nc.sync.dma_start(
                        out=pt[:],
                        in_=bass.AP(prf.tensor, t * G * A,
                                    [[TG_ * A, P], [A, G], [1, A]]))

                    # padded p_eff (bf16), single two-window pad memset
                    PB = wk2.tile([P, G * WPB], BF16, tag="PB")

                    def pbv(off, w):
                        h = PB[:]
                        return bass.AP(h.tensor, h.offset + off,
                                       [h.ap[0], [WPB, G], [1, w]])

                    pbh = PB[:]
                    nc.gpsimd.memset(
                        bass.AP(pbh.tensor, pbh.offset,
                                [pbh.ap[0], [WPB, G], [PADL + A, 2],
                                 [1, PADL]]), 0.0)
                    nc.gpsimd.tensor_tensor(
                        out=pbv(PADL, A), in0=pt[:], in1=gview(mfv, t, A),
                        op=OP.mult)
                    pbcol = bass.AP(pbh.tensor, pbh.offset + PADL + 25,
                                    [pbh.ap[0], [WPB, G]])
                    nc.gpsimd.tensor_tensor(
                        out=pbcol, in0=pbcol,
                        in1=bass.AP(nm16[:].tensor, nm16[:].offset + t * G,
                                    [nm16[:].ap[0], [1, G]]), op=OP.add)

                    # tap weights: au3 = relu(1 - |rho - 0.01 j + g k|),
                    # all 3 sections in one op pair
                    au3 = wk1.tile([P, G * W3], BF16, tag="au3")
                    nc.vector.tensor_tensor(
                        out=au3[:], in0=gview(rho, t, W3),
                        in1=bass.AP(j3[:].tensor, j3[:].offset,
                                    [j3[:].ap[0], [0, G], [1, W3]]),
                        op=OP.add)
                    nc.scalar.activation(out=au3[:], in_=au3[:], func=AF.Abs,
                                         bias=0.0, scale=1.0)
                    nc.scalar.activation(out=au3[:], in_=au3[:], func=AF.Relu,
                                         bias=bone[:], scale=-1.0)

                    def auv(k):
                        h = au3[:]
                        return bass.AP(h.tensor, h.offset + (k + 1) * A,
                                       [h.ap[0], [W3, G], [1, A]])

                    # coarse select: T1[x] = PB[x + 5c]
                    T1 = wk1.tile([P, G * WT1S], BF16, tag="T1")

                    def t1v(off, w):
                        h = T1[:]
                        return bass.AP(h.tensor, h.offset + off,
                                       [h.ap[0], [WT1S, G], [1, w]])

                    nc.vector.tensor_copy(out=t1v(0, WT1), in_=pbv(0, WT1))
                    for c in range(1, 6):
                        nc.vector.copy_predicated(
                            out=t1v(0, WT1), mask=gview(mcs[c], t, WT1),
                            data=pbv(5 * c, WT1))
                    # fine select: pS[x] = T1[x + f]
                    pS = wk1.tile([P, G * WPSS], BF16, tag="pS")

                    def psv(off, w):
                        h = pS[:]
                        return bass.AP(h.tensor, h.offset + off,
                                       [h.ap[0], [WPSS, G], [1, w]])

                    nc.vector.tensor_copy(out=psv(0, WPS), in_=t1v(0, WPS))
                    for f in range(1, 5):
                        nc.vector.copy_predicated(
                            out=psv(0, WPS), mask=gview(mfs[f], t, WPS),
                            data=t1v(f, WPS))

                    # taps (bf16 accumulate, final add widens to f32)
                    mt = iop.tile([P, FA], F32, tag="mt")
                    tm1 = wk1.tile([P, FA], BF16, tag="tm1")
                    tm2 = wk1.tile([P, FA], BF16, tag="tm2")
                    nc.vector.tensor_tensor(out=tm1[:], in0=auv(-1),
                                            in1=psv(0, A), op=OP.mult)
                    nc.vector.tensor_tensor(out=tm2[:], in0=auv(0),
                                            in1=psv(1, A), op=OP.mult)
                    nc.vector.tensor_tensor(out=tm1[:], in0=tm1[:],
                                            in1=tm2[:], op=OP.add)
                    nc.vector.tensor_tensor(out=tm2[:], in0=auv(1),
                                            in1=psv(2, A), op=OP.mult)
                    nc.vector.tensor_tensor(out=mt[:], in0=tm1[:],
                                            in1=tm2[:], op=OP.add)

                    # edges: both windows in one op chain; PB already
                    # carries p*mask (one-hot col 42 is outside windows)
                    wc = wk2.tile([P, G * 2 * EW], BF16, tag="wc")

                    def wcv(dims):
                        h = wc[:]
                        return bass.AP(h.tensor, h.offset, [h.ap[0]] + dims)

                    nc.gpsimd.tensor_tensor(
                        out=wcv([[2 * EW, G], [1, 2 * EW]]),
                        in0=bass.AP(tabLR[:].tensor, tabLR[:].offset,
                                    [tabLR[:].ap[0], [0, G], [1, 2 * EW]]),
                        in1=bass.AP(pnq[:].tensor, pnq[:].offset + t * 2 * G,
                                    [pnq[:].ap[0], [2, G], [1, 2], [0, EW]]),
                        op=OP.add)
                    nc.gpsimd.tensor_scalar(out=wc[:], in0=wc[:], scalar1=0.0,
                                            scalar2=1.0, op0=OP.max,
                                            op1=OP.min)
                    nc.gpsimd.tensor_tensor(
                        out=wcv([[2 * EW, G], [1, 2 * EW]]),
                        in0=wcv([[2 * EW, G], [1, 2 * EW]]),
                        in1=bass.AP(pbh.tensor, pbh.offset + PADL,
                                    [pbh.ap[0], [WPB, G], [35, 2], [1, EW]]),
                        op=OP.mult)
                    m05 = wk2.tile([P, G * 2], F32, tag="m05")
                    nc.vector.tensor_reduce(
                        out=bass.AP(m05[:].tensor, m05[:].offset,
                                    [m05[:].ap[0], [2, G], [1, 2]]),
                        in_=wcv([[2 * EW, G], [EW, 2], [1, EW]]),
                        axis=mybir.AxisListType.X, op=OP.add)
                    mth = mt[:]
                    nc.gpsimd.tensor_copy(
                        out=bass.AP(mth.tensor, mth.offset,
                                    [mth.ap[0], [A, G], [50, 2]]),
                        in_=bass.AP(m05[:].tensor, m05[:].offset,
                                    [m05[:].ap[0], [2, G], [1, 2]]))

                    nc.sync.dma_start(
                        out=bass.AP(mof.tensor, t * G * A,
                                    [[TG_ * A, P], [A, G], [1, A]]),
                        in_=mt[:])
    nc.compile()
    return nc


_NC_CACHE = {}


def kernel(batch_reward, max_next_dist, supports, non_final_mask):
    assert max_next_dist.shape == (B_TOTAL, A)
    if "nc" not in _NC_CACHE:
        _NC_CACHE["nc"] = _build_nc(BC)
    nc = _NC_CACHE["nc"]
    j3, tabLR = _host_consts()
    in_maps = []
    for c in range(N_CORES):
        s = slice(c * BC, (c + 1) * BC)
        in_maps.append({
            "pdist": np.ascontiguousarray(max_next_dist[s]).astype(np.float32),
            "reward": np.ascontiguousarray(batch_reward[s]).astype(np.float32),
            "mask": np.ascontiguousarray(non_final_mask[s]).astype(np.int32),
            "j3": j3, "tabLR": tabLR,
        })
    res = run_bass_kernel_spmd(nc, in_maps, core_ids=list(range(N_CORES)))
    return np.concatenate([res.results[c]["mout"] for c in range(N_CORES)],
                          axis=0)
